# revision 2
# baseline (speedup 1.0000x reference)
"""Trainium2 Bass kernel for the GRU encoder problem (nn_Encoder).

Computation: x = embedding[source]; gi = x @ w_ih.T + b_ih; then a GRU
recurrence over T=128 steps producing enc_outputs [T, B, H].

Strategy: data-parallel over batch across 8 NeuronCores (B=64 -> 8 rows/core,
embedding + GRU weights replicated, all in bf16 converted host-side). Inside
each core everything runs in a "transposed" layout with gate/hidden dims on
SBUF partitions and batch on the free dim:

  phase A: dma_gather   x[tok, :] = embedding[source[tok]]   (tok t-major)
  phase B: PE-transpose x -> xT (bf16)
  phase C: load w_ih.T / w_hh.T bf16 directly (no on-chip conversion)
  phase D: gi GEMM in 8 token-chunks of 128 (= 16 time steps each).
           Chunks 0-1 run up front; chunks 2-7 are interleaved into the
           recurrence 16 matmuls per step to fill PE idle windows.
  phase E: recurrence, per step t:
             PSUM G_rz prefilled with gi_rz, G_n with b_hh_n (DVE), then
             G += whh_k^T @ h_k accumulated with start=False.
             rz = sigmoid(G_rz); omz = sigmoid(-G_rz[z])
             n  = tanh(r * G_n' + gi_n)
             h' = omz * n + z * h        -> outT[t]
           Tail split across ACT/DVE/Pool engines to shorten the chain.
The h' tile in layout [128, k*8+b] is exactly the next step's matmul rhs, so
no per-step transposes are needed.
"""
import numpy as np

V, E, H, B, T = 32000, 1024, 1280, 64, 128
BL = 8            # batch rows per core
G3 = 3 * H        # 3840
NJ = G3 // 128    # 30 gate blocks
NK = H // 128     # 10 hidden blocks
NE = E // 128     # 8 embedding blocks
N_CORES = 8
CH = 16           # time steps per gi chunk
NCH = T // CH     # 8 chunks
TOKC = CH * BL    # 128 tokens per chunk
UPF = 2           # chunks computed before the recurrence starts
MMS_PER_STEP = 16  # interleaved gemm matmuls pulled per recurrence step

_CACHE = {}


def _build(nc):
    import concourse.mybir as mybir
    import concourse.tile as tile

    F32 = mybir.dt.float32
    BF16 = mybir.dt.bfloat16
    I16 = mybir.dt.int16
    AF = mybir.ActivationFunctionType
    OP = mybir.AluOpType

    NTOK = T * BL
    NTC = NTOK // 128

    emb_d = nc.dram_tensor("emb", [V, E], BF16, kind="ExternalInput")
    idx_d = nc.dram_tensor("idx", [128, NTOK // 16], I16, kind="ExternalInput")
    wihT_d = nc.dram_tensor("wihT", [E, G3], BF16, kind="ExternalInput")
    whhT_d = nc.dram_tensor("whhT", [H, G3], BF16, kind="ExternalInput")
    bgi_d = nc.dram_tensor("bgi", [128, NJ], F32, kind="ExternalInput")
    bhhn_d = nc.dram_tensor("bhhn", [128, BL * NK], F32, kind="ExternalInput")
    ident_d = nc.dram_tensor("ident", [128, 128], BF16, kind="ExternalInput")
    giT_d = nc.dram_tensor("giT", [128, NJ, NTOK], BF16, kind="Internal")
    outT_d = nc.dram_tensor("outT", [T, 128, BL * NK], F32, kind="ExternalOutput")

    with tile.TileContext(nc) as tc:
        with tc.tile_pool(name="const", bufs=1) as cpool:
            bgi = cpool.tile([128, NJ], F32, tag="bgi")
            nc.sync.dma_start(bgi[:, :], bgi_d.ap())
            bhhn = cpool.tile([128, BL * NK], F32, tag="bhhn")
            nc.sync.dma_start(bhhn[:, :], bhhn_d.ap())
            ident = cpool.tile([128, 128], BF16, tag="ident")
            nc.sync.dma_start(ident[:, :], ident_d.ap())
            idx_sb = cpool.tile([128, NTOK // 16], I16, tag="idx")
            nc.sync.dma_start(idx_sb[:, :], idx_d.ap())

            # resident bf16 weights (direct DMA, no conversion)
            wih = cpool.tile([128, NE, G3], BF16, tag="wih")
            for e in range(NE):
                nc.sync.dma_start(
                    wih[:, e, :], wihT_d.ap()[128 * e:128 * (e + 1), :])
            whh = cpool.tile([128, NK, G3], BF16, tag="whh")
            for k in range(NK):
                nc.sync.dma_start(
                    whh[:, k, :], whhT_d.ap()[128 * k:128 * (k + 1), :])

            # ---------- phases A-B: gather + transpose ----------
            xT = cpool.tile([128, NE, NTOK], BF16, tag="xT")
            with tc.tile_pool(name="xp", bufs=1) as xp:
                x_sb = xp.tile([128, NTC, E], BF16, tag="x")
                nc.gpsimd.dma_gather(
                    x_sb[:, :, :], emb_d.ap(), idx_sb[:, :],
                    num_idxs=NTOK, num_idxs_reg=NTOK, elem_size=E)
                with tc.tile_pool(name="trps", bufs=4, space="PSUM") as tpp:
                    for c in range(NTC):
                        for e in range(NE):
                            tps = tpp.tile([128, 128], F32, tag="tps")
                            nc.tensor.transpose(
                                tps[:, :], x_sb[:, c, 128 * e:128 * (e + 1)],
                                ident[:, :])
                            nc.vector.tensor_copy(
                                xT[:, e, 128 * c:128 * (c + 1)], tps[:, :])

            # ---------- phase D: gi GEMM as chunk op-list ----------
            gp_ctx = tc.tile_pool(name="gemm", bufs=1)
            gp = gp_ctx.__enter__()
            gpp_ctx = tc.tile_pool(name="gips", bufs=4, space="PSUM")
            gpp = gpp_ctx.__enter__()
            gstate = {}

            def mm_op(c, j, e):
                def f():
                    if e == 0:
                        gstate["gps"] = gpp.tile([128, TOKC], F32, tag="gps")
                    gps = gstate["gps"]
                    nc.tensor.matmul(
                        gps[:, :],
                        wih[:, e, 128 * j:128 * (j + 1)],
                        xT[:, e, TOKC * c:TOKC * (c + 1)],
                        start=(e == 0), stop=(e == NE - 1))
                    if e == NE - 1:
                        gsb = gp.tile([128, TOKC], BF16, tag="gsb", bufs=3)
                        nc.scalar.activation(
                            gsb[:, :], gps[:, :], AF.Identity,
                            bias=bgi[:, j:j + 1])
                        nc.sync.dma_start(
                            giT_d.ap()[:, j, TOKC * c:TOKC * (c + 1)],
                            gsb[:, :])
                return f

            # up-front chunks
            for c in range(UPF):
                for j in range(NJ):
                    for e in range(NE):
                        mm_op(c, j, e)()
            # deferred ops, pulled during the recurrence
            gemm_ops = [mm_op(c, j, e)
                        for c in range(UPF, NCH)
                        for j in range(NJ)
                        for e in range(NE)]
            gemm_pos = [0]

            def pull_gemm(nops):
                lo = gemm_pos[0]
                hi = min(lo + nops, len(gemm_ops))
                for i in range(lo, hi):
                    gemm_ops[i]()
                gemm_pos[0] = hi

            # ---------- phase E: recurrence ----------
            with tc.tile_pool(name="recw", bufs=4) as rw, \
                 tc.tile_pool(name="gicp", bufs=2) as gip, \
                 tc.tile_pool(name="hpool", bufs=3) as hp, \
                 tc.tile_pool(name="grz_ps", bufs=2, space="PSUM") as rzp, \
                 tc.tile_pool(name="gn_ps", bufs=2, space="PSUM") as gnp:
                h = hp.tile([128, BL * NK], F32, tag="h")
                nc.vector.memset(h[:, :], 0.0)
                HB = BL * NK // 2          # 40: half of the hc free dim
                hc_a = hp.tile([128, HB], BF16, tag="hca")
                nc.vector.memset(hc_a[:, :], 0.0)
                hc_b = hp.tile([128, HB], BF16, tag="hcb")
                nc.vector.memset(hc_b[:, :], 0.0)

                gic = [None] * NCH
                gic[0] = gip.tile([128, NJ, TOKC], BF16, tag="gic")
                nc.sync.dma_start(gic[0][:, :, :],
                                  giT_d.ap()[:, :, 0:TOKC])

                # first step's PSUM prefill
                G_rz = rzp.tile([128, 20 * BL], F32, tag="grz")
                G_n = gnp.tile([128, 10 * BL], F32, tag="gn")
                nc.vector.tensor_copy(
                    G_rz[:, :],
                    gic[0][:, 0:20, 0:BL].rearrange("p j b -> p (j b)"))
                nc.vector.tensor_copy(G_n[:, :], bhhn[:, :])

                for t in range(T):
                    c, ci = divmod(t, CH)
                    if ci == 0 and c + 1 < NCH:
                        gic[c + 1] = gip.tile([128, NJ, TOKC], BF16, tag="gic")
                        nc.sync.dma_start(
                            gic[c + 1][:, :, :],
                            giT_d.ap()[:, :, TOKC * (c + 1):TOKC * (c + 2)])

                    def gate_mms(jlo, jhi, sl_of):
                        for j in range(jlo, jhi):
                            sl = sl_of(j)
                            for k in range(NK):
                                hc_h = hc_a if k < 5 else hc_b
                                nc.tensor.matmul(
                                    sl,
                                    whh[:, k, 128 * j:128 * (j + 1)],
                                    hc_h[:, BL * (k % 5):BL * (k % 5 + 1)],
                                    start=False, stop=(k == NK - 1),
                                    skip_group_check=True)

                    # r and z gate blocks (j 0..19) accumulate onto gi prefill
                    gate_mms(0, 20, lambda j: G_rz[:, BL * j:BL * (j + 1)])
                    rz = rw.tile([128, 20 * BL], F32, tag="rz")
                    nc.scalar.activation(rz[:, :], G_rz[:, :], AF.Sigmoid)
                    omz = rw.tile([128, 10 * BL], F32, tag="omz")
                    nc.scalar.activation(
                        omz[:, :], G_rz[:, 10 * BL:20 * BL], AF.Sigmoid,
                        scale=-1.0)
                    # z*h on Pool, parallel with the DVE n-path
                    zh = rw.tile([128, 10 * BL], F32, tag="zh")
                    nc.gpsimd.tensor_tensor(
                        zh[:, :], rz[:, 10 * BL:20 * BL], h[:, :], OP.mult)

                    # n gate blocks (j 20..29) accumulate onto b_hh_n prefill
                    gate_mms(20, 30,
                             lambda j: G_n[:, BL * (j - 20):BL * (j - 19)])
                    t2 = rw.tile([128, 10 * BL], F32, tag="t2")
                    nc.vector.tensor_tensor(
                        t2[:, :], G_n[:, :], rz[:, 0:10 * BL], OP.mult)
                    npre = rw.tile([128, 10 * BL], F32, tag="npre")
                    nc.vector.tensor_tensor(
                        npre[:, :], t2[:, :],
                        gic[c][:, 20:30, BL * ci:BL * (ci + 1)]
                        .rearrange("p j b -> p (j b)"), OP.add)
                    n_g = rw.tile([128, 10 * BL], F32, tag="ng")
                    nc.scalar.activation(n_g[:, :], npre[:, :], AF.Tanh)

                    # next step's PSUM prefill (other buffer), off the chain
                    if t + 1 < T:
                        c2, ci2 = divmod(t + 1, CH)
                        G_rz2 = rzp.tile([128, 20 * BL], F32, tag="grz")
                        G_n2 = gnp.tile([128, 10 * BL], F32, tag="gn")
                        nc.vector.tensor_copy(
                            G_rz2[:, :],
                            gic[c2][:, 0:20, BL * ci2:BL * (ci2 + 1)]
                            .rearrange("p j b -> p (j b)"))
                        nc.vector.tensor_copy(G_n2[:, :], bhhn[:, :])

                    # h' = omz*n + zh ; bf16 halves on Pool feed next matmuls
                    m = rw.tile([128, 10 * BL], F32, tag="m")
                    nc.vector.tensor_tensor(
                        m[:, :], omz[:, :], n_g[:, :], OP.mult)
                    hc_a = hp.tile([128, HB], BF16, tag="hca")
                    nc.gpsimd.tensor_tensor(
                        hc_a[:, :], m[:, 0:HB], zh[:, 0:HB], OP.add)
                    hc_b = hp.tile([128, HB], BF16, tag="hcb")
                    nc.gpsimd.tensor_tensor(
                        hc_b[:, :], m[:, HB:2 * HB], zh[:, HB:2 * HB], OP.add)
                    h = hp.tile([128, BL * NK], F32, tag="h")
                    nc.vector.tensor_tensor(
                        h[:, :], m[:, :], zh[:, :], OP.add)
                    nc.sync.dma_start(outT_d.ap()[t, :, :], h[:, :])
                    if t + 1 < T:
                        G_rz, G_n = G_rz2, G_n2

                    pull_gemm(MMS_PER_STEP)

            gpp_ctx.__exit__(None, None, None)
            gp_ctx.__exit__(None, None, None)


class _Compiled:
    def __init__(self):
        import jax
        import numpy as _np
        import concourse.bacc as bacc
        import concourse.mybir as mybir
        from jax.sharding import Mesh, PartitionSpec, NamedSharding
        from jax.experimental.shard_map import shard_map
        from concourse.bass2jax import (
            _bass_exec_p, partition_id_tensor, install_neuronx_cc_hook)

        install_neuronx_cc_hook()
        nc = bacc.Bacc("TRN2", target_bir_lowering=False, debug=False,
                       enable_asserts=True, num_devices=1)
        _build(nc)
        nc.compile()
        self.nc = nc
        self.jax = jax

        partition_name = (nc.partition_id_tensor.name
                          if nc.partition_id_tensor else None)
        in_names, out_names, out_avals, zero_outs = [], [], [], []
        for alloc in nc.m.functions[0].allocations:
            if not isinstance(alloc, mybir.MemoryLocationSet):
                continue
            name = alloc.memorylocations[0].name
            if alloc.kind == "ExternalInput":
                if name != partition_name:
                    in_names.append(name)
            elif alloc.kind == "ExternalOutput":
                out_names.append(name)
                shape = tuple(alloc.tensor_shape)
                dt = mybir.dt.np(alloc.dtype)
                out_avals.append(jax.core.ShapedArray(shape, dt))
                zero_outs.append(_np.zeros(shape, dt))
        self.in_params = list(in_names)
        self.out_names = out_names
        self.out_avals = out_avals
        n_params = len(in_names)
        in_names = in_names + out_names
        if partition_name is not None:
            in_names.append(partition_name)

        def _body(*args):
            args = list(args)
            if partition_name is not None:
                args.append(partition_id_tensor())
            outs = _bass_exec_p.bind(
                *args, out_avals=tuple(out_avals), in_names=tuple(in_names),
                out_names=tuple(out_names), lowering_input_output_aliases=(),
                sim_require_finite=True, sim_require_nnan=True, nc=nc)
            return tuple(outs)

        devices = jax.devices()[:N_CORES]
        mesh = Mesh(_np.asarray(devices), ("core",))
        n_in = n_params + len(out_names)
        self.sharded = jax.jit(
            shard_map(_body, mesh=mesh,
                      in_specs=(PartitionSpec("core"),) * n_in,
                      out_specs=(PartitionSpec("core"),) * len(out_names),
                      check_rep=False),
            keep_unused=True)
        self.sh = NamedSharding(mesh, PartitionSpec("core"))
        self.zero_outs = zero_outs

    def put_inputs(self, in_maps):
        import numpy as _np
        jax = self.jax
        concat = [_np.concatenate([_np.ascontiguousarray(in_maps[c][n])
                                   for c in range(N_CORES)], axis=0)
                  for n in self.in_params]
        args = [jax.device_put(a, self.sh) for a in concat]
        zeros = [jax.device_put(
            _np.zeros((N_CORES * z.shape[0], *z.shape[1:]), z.dtype), self.sh)
            for z in self.zero_outs]
        return args + zeros

    def run(self, dev_args):
        out = self.sharded(*dev_args)
        self.jax.block_until_ready(out)
        return out

    def results(self, out):
        import numpy as _np
        res = []
        for c in range(N_CORES):
            d = {}
            for i, name in enumerate(self.out_names):
                a = _np.asarray(out[i])
                d[name] = a.reshape(N_CORES, *self.out_avals[i].shape)[c]
            res.append(d)
        return res


def _get_compiled():
    if "k" not in _CACHE:
        _CACHE["k"] = _Compiled()
    return _CACHE["k"]


def _prep_core_inputs(source_core, embedding, wihT, whhT, bgi, bhhn, ident):
    NTOK = T * BL
    idx_lin = source_core.T.reshape(-1)          # t-major: i = t*8 + b
    idx = np.tile(idx_lin.reshape(NTOK // 16, 16).T, (8, 1)).astype(np.int16)
    return {"emb": embedding, "idx": idx, "wihT": wihT, "whhT": whhT,
            "bgi": bgi, "bhhn": bhhn, "ident": ident}


def prep_in_maps(source, embedding, w_ih, w_hh, b_ih, b_hh):
    import ml_dtypes
    bf16 = ml_dtypes.bfloat16
    source = np.asarray(source)
    embedding = np.ascontiguousarray(
        np.asarray(embedding, dtype=np.float32).astype(bf16))
    w_ih = np.asarray(w_ih, dtype=np.float32)
    w_hh = np.asarray(w_hh, dtype=np.float32)
    b_ih = np.asarray(b_ih, dtype=np.float32)
    b_hh = np.asarray(b_hh, dtype=np.float32)
    wihT = np.ascontiguousarray(w_ih.T.astype(bf16))
    whhT = np.ascontiguousarray(w_hh.T.astype(bf16))
    bias_gi = np.concatenate([(b_ih + b_hh)[:2 * H], b_ih[2 * H:]])
    bgi = np.ascontiguousarray(bias_gi.reshape(NJ, 128).T, dtype=np.float32)
    bhh_n = b_hh[2 * H:]
    bhhn = np.ascontiguousarray(
        np.repeat(bhh_n.reshape(NK, 128).T[:, :, None], BL, axis=2)
        .reshape(128, NK * BL), dtype=np.float32)
    ident = np.eye(128, dtype=np.float32).astype(bf16)
    return [
        _prep_core_inputs(source[c * BL:(c + 1) * BL], embedding, wihT, whhT,
                          bgi, bhhn, ident)
        for c in range(N_CORES)]


def unpack_results(res):
    """res: list of per-core {'outT': [T, 128, 80]} -> [T, B, H] float32."""
    outs = []
    for c in range(N_CORES):
        o = res[c]["outT"].reshape(T, 128, NK, BL)
        outs.append(o.transpose(0, 3, 2, 1).reshape(T, BL, H))
    return np.concatenate(outs, axis=1).astype(np.float32)


def kernel(source, embedding, w_ih, w_hh, b_ih, b_hh):
    k = _get_compiled()
    in_maps = prep_in_maps(source, embedding, w_ih, w_hh, b_ih, b_hh)
    dev_args = k.put_inputs(in_maps)
    out = k.run(dev_args)
    return unpack_results(k.results(out))
